# revision 1
# baseline (speedup 1.0000x reference)
# Trainium2 Bass kernel for DMOR (dynamic mixture-of-operators routing).
#
# Reference computation (per image):
#   op_feats = [x, conv3x3(x), conv3x3_dilated2(x), avgpool3x3(x), dwconv3x3(x)]
#   z = spatial_router(x) + global_router(GAP(x))          # [5, H, W]
#   w = softmax(z, axis=0); top-2 mask + renormalize (eps=1e-6)
#   out = sum_n w_n * op_feats_n
#
# Sharding: data-parallel over batch B=8 across 8 NeuronCores (1 image/core),
# weights replicated. One SPMD program; per-core in_maps differ only in x.
#
# Device structure (per core, x:[64,128,128] -> out:[64,128,128]):
#   - All stencil ops become 9-shift accumulating PSUM matmuls in float32r
#     (~1.7e-4 rel err, ~1 col/cycle). fp32r tolerates only plain full-array
#     matmuls (no tile_position), so shift-pairs are packed into K=128 by
#     storing x twice: partitions 0-63 hold padded x, partitions 64-127 hold
#     the same image shifted left by 2 columns. 3 PSUM groups:
#       bankA[128,512] = [conv3x3 | avgpool]   (avg = (1/9)*I diagonal)
#       bankBX[128,512] = [dwconv | identity]  (identity = I at center shift)
#       bankC[64,512]  = dilated conv
#   - Router z: exact fp32 matmuls (top-2 selection is discontinuous; z gaps
#     go down to ~1e-7). GAP via an ACT copy pass with accum_out; tiny MLP on
#     the PE; the data-dependent global bias is fused into the z drain.
#   - z is moved to a pixel-major layout [128,5,128] via a DRAM bounce
#     (partition-crossing SBUF->SBUF DMAs are miscompiled, measured), then
#     softmax/top-2 runs as ~13 whole-image DVE/ACT ops.
#   - w returns flat via DRAM; K=5 ones-matmuls broadcast the three needed
#     (map-pair) layouts across channel partitions; the mix is 3 fused
#     (f + bias) * w ops per tile; the 5-term sum accumulates in PSUM via
#     identity matmuls.
import numpy as np

B, C, H, W = 8, 64, 128, 128
HW = H * W
N_OPS = 5
HID = 16
PAD = 2
HP, WP = H + 2 * PAD, W + 2 * PAD  # 132, 132
TPX = 512                          # pixels per tile (4 image rows)
NT = HW // TPX                     # 32 tiles
ROWS_PER_TILE = TPX // W           # 4
EPS = 1e-6
NEG_BIG = -1.0e30

_CACHE = {}


def _host_consts(w3, b3, wd, bd, wdw, bdw, gr_w1, gr_w2, gr_b2, sr_w, sr_b):
    """Precompute all stationary matmul operands / bias vectors in numpy.

    Matmul i in 0..5 per op group: i<3 are K=128 shift-pairs (kh=i, partition
    half 0 = col-shift kw0, half 1 = kw1 via the +2-shifted x copy); i>=3 are
    K=64 unpaired (kh=i-3, remaining kw)."""
    f32 = np.float32
    w3 = np.asarray(w3, f32); wd = np.asarray(wd, f32); wdw = np.asarray(wdw, f32)
    eye = np.eye(C, dtype=f32)
    avg = eye * f32(1.0 / 9.0)

    A = np.zeros((128, 6, 128), f32)
    Bd = np.zeros((128, 6, 128), f32)
    Cd = np.zeros((128, 6, 64), f32)
    for kh in range(3):
        # paired: conv3/dw cols (kw=0 | kw=2); dilated cols (kw=0 | kw=1)
        A[0:64, kh, 0:64] = w3[:, :, kh, 0].T
        A[64:128, kh, 0:64] = w3[:, :, kh, 2].T
        A[0:64, kh, 64:128] = avg
        A[64:128, kh, 64:128] = avg
        Bd[0:64, kh, 0:64] = eye * wdw[:, 0, kh, 0][None, :]
        Bd[64:128, kh, 0:64] = eye * wdw[:, 0, kh, 2][None, :]
        Cd[0:64, kh, :] = wd[:, :, kh, 0].T
        Cd[64:128, kh, :] = wd[:, :, kh, 1].T
        # unpaired: conv3/dw kw=1; dilated kw=2
        A[0:64, 3 + kh, 0:64] = w3[:, :, kh, 1].T
        A[0:64, 3 + kh, 64:128] = avg
        Bd[0:64, 3 + kh, 0:64] = eye * wdw[:, 0, kh, 1][None, :]
        Cd[0:64, 3 + kh, :] = wd[:, :, kh, 2].T
    Bd[0:64, 4, 64:128] = eye  # identity op = center shift (kh=1, kw=1)

    R = np.ascontiguousarray(np.asarray(sr_w, f32)[:, :, 0, 0].T)          # [C, 5]
    G1 = np.ascontiguousarray((np.asarray(gr_w1, f32)[:, :, 0, 0] / HW).T)  # [C, HID]
    G2 = np.ascontiguousarray(np.asarray(gr_w2, f32)[:, :, 0, 0].T)        # [HID, 5]
    HBIAS = (np.asarray(sr_b, f32) + np.asarray(gr_b2, f32)).reshape(N_OPS, 1)

    SUM128 = np.ascontiguousarray(np.concatenate([eye, eye], axis=0))  # [128, 64]

    def bc(n_lo, n_hi=None):
        m = 128 if n_hi is not None else 64
        out = np.zeros((N_OPS, m), f32)
        out[n_lo, 0:64] = 1.0
        if n_hi is not None:
            out[n_hi, 64:128] = 1.0
        return out

    BIAS13 = np.concatenate([np.asarray(b3, f32), np.zeros(C, f32)]).reshape(128, 1)
    BIAS40 = np.concatenate([np.asarray(bdw, f32), np.zeros(C, f32)]).reshape(128, 1)
    BIAS2 = np.asarray(bd, f32).reshape(C, 1)

    return {
        "cA": np.ascontiguousarray(A.reshape(128, 6 * 128)),
        "cB": np.ascontiguousarray(Bd.reshape(128, 6 * 128)),
        "cC": np.ascontiguousarray(Cd.reshape(128, 6 * 64)),
        "cR": R, "cG1": G1, "cG2": G2, "cHB": HBIAS,
        "cSUM128": SUM128, "cSUM64": eye,
        "cBC13": bc(1, 3), "cBC40": bc(4, 0), "cBC2": bc(2),
        "cBIAS13": BIAS13, "cBIAS40": BIAS40, "cBIAS2": BIAS2,
        "cZ": np.zeros((128, 544), f32),
    }


def _build_program(nrep=1):
    import concourse.bass as bass
    import concourse.bacc as bacc
    import concourse.tile as tile
    import concourse.mybir as mybir
    from contextlib import ExitStack

    dt = mybir.dt
    f32 = dt.float32
    f32r = dt.float32r
    AF = mybir.ActivationFunctionType
    ALU = mybir.AluOpType
    AX = mybir.AxisListType

    nc = bacc.Bacc("TRN2", target_bir_lowering=False, debug=False)

    xin = nc.dram_tensor("xin", [C, HW], f32, kind="ExternalInput")
    dr = {}
    for name, shape in [
        ("cA", [128, 6 * 128]), ("cB", [128, 6 * 128]), ("cC", [128, 6 * 64]),
        ("cR", [C, N_OPS]), ("cG1", [C, HID]), ("cG2", [HID, N_OPS]),
        ("cHB", [N_OPS, 1]), ("cSUM128", [128, 64]), ("cSUM64", [64, 64]),
        ("cBC13", [N_OPS, 128]), ("cBC40", [N_OPS, 128]), ("cBC2", [N_OPS, 64]),
        ("cBIAS13", [128, 1]), ("cBIAS40", [128, 1]), ("cBIAS2", [64, 1]),
        ("cZ", [128, 544]),
    ]:
        dr[name] = nc.dram_tensor(name, shape, f32, kind="ExternalInput")
    yout = nc.dram_tensor("yout", [C, HW], f32, kind="ExternalOutput")

    def b5(t):
        # broadcast a [128,128] tile across the 5-map free dim -> [128,5,128]
        return bass.AP(tensor=t.tensor, offset=t.offset,
                       ap=[list(t.ap[0]), [0, N_OPS], list(t.ap[1])])

    def rr(ap):
        return ap.bitcast(f32r)

    with tile.TileContext(nc) as tc, ExitStack() as ctx:
        consts = ctx.enter_context(tc.tile_pool(name="consts", bufs=1))
        xpool = ctx.enter_context(tc.tile_pool(name="xp", bufs=1))
        zpool = ctx.enter_context(tc.tile_pool(name="z", bufs=1))
        gaps = ctx.enter_context(tc.tile_pool(name="gaps", bufs=1))
        zchunk = ctx.enter_context(tc.tile_pool(name="zchunk", bufs=2))
        wchunk = ctx.enter_context(tc.tile_pool(name="wchunk", bufs=3))
        drpool = ctx.enter_context(tc.tile_pool(name="drbounce", bufs=1,
                                                space="DRAM"))

        # ---- constant tiles -------------------------------------------------
        wA = consts.tile([128, 6, 128], f32r)
        wB = consts.tile([128, 6, 128], f32r)
        wC = consts.tile([128, 6, 64], f32r)
        s128 = consts.tile([128, 64], f32r)
        s64 = consts.tile([C, 64], f32r)
        bc13 = consts.tile([N_OPS, 128], f32r)
        bc40 = consts.tile([N_OPS, 128], f32r)
        bc2 = consts.tile([N_OPS, 64], f32r)
        wR = consts.tile([C, N_OPS], f32)
        wG1 = consts.tile([C, HID], f32)
        wG2 = consts.tile([HID, N_OPS], f32)
        hbv = consts.tile([N_OPS, 1], f32)
        bias13 = consts.tile([128, 1], f32)
        bias40 = consts.tile([128, 1], f32)
        bias2 = consts.tile([C, 1], f32)

        nc.sync.dma_start(out=wA,
                          in_=rr(dr["cA"][:, :]).rearrange("c (s m) -> c s m", s=6))
        nc.sync.dma_start(out=wB,
                          in_=rr(dr["cB"][:, :]).rearrange("c (s m) -> c s m", s=6))
        nc.sync.dma_start(out=wC,
                          in_=rr(dr["cC"][:, :]).rearrange("c (s m) -> c s m", s=6))
        nc.sync.dma_start(out=s128, in_=rr(dr["cSUM128"][:, :]))
        nc.sync.dma_start(out=s64, in_=rr(dr["cSUM64"][:, :]))
        nc.sync.dma_start(out=bc13, in_=rr(dr["cBC13"][:, :]))
        nc.sync.dma_start(out=bc40, in_=rr(dr["cBC40"][:, :]))
        nc.sync.dma_start(out=bc2, in_=rr(dr["cBC2"][:, :]))
        for t, name in [(wR, "cR"), (wG1, "cG1"), (wG2, "cG2"), (hbv, "cHB"),
                        (bias13, "cBIAS13"), (bias40, "cBIAS40"),
                        (bias2, "cBIAS2")]:
            nc.sync.dma_start(out=t, in_=dr[name][:, :])

        zpm = zpool.tile([128, N_OPS, 128], f32)
        rep_pools = (consts, xpool, zpool, gaps, zchunk, wchunk, drpool)
        gscr = gaps.tile([C, 2048], f32)
        gparts = gaps.tile([C, 8], f32)
        xg = gaps.tile([C, 1], f32)
        hrelu = gaps.tile([HID, 1], f32)
        biasv = gaps.tile([N_OPS, 1], f32)
        zscr = drpool.tile([N_OPS, HW], f32)
        wscr = drpool.tile([N_OPS, HW], f32)


        m1 = zpool.tile([128, 128], f32)
        m2 = zpool.tile([128, 128], f32)
        zsum = zpool.tile([128, 128], f32)
        e2 = zpool.tile([128, 128], f32)
        eqx = zpool.tile([128, N_OPS, 128], f32)
        em = zpool.tile([128, N_OPS, 128], f32)
        wpm = zpool.tile([128, N_OPS, 128], f32)

        def overn(t):
            return t[:, :, :].rearrange("p n b -> p b n")

        for _rep in range(nrep):
            # -- fp32r x, two copies: half0 = padded x, half1 = shifted left 2
            xpr = xpool.tile([128, HP, WP], f32r)
            zsrc = rr(dr["cZ"][:, :])
            nc.sync.dma_start(out=xpr[:, 0:PAD, :],
                              in_=zsrc[:, 0:PAD * WP]
                              .rearrange("p (a b) -> p a b", a=PAD))
            nc.sync.dma_start(out=xpr[:, HP - PAD:HP, :],
                              in_=zsrc[:, 0:PAD * WP]
                              .rearrange("p (a b) -> p a b", a=PAD))
            nc.sync.dma_start(out=xpr[0:64, PAD:HP - PAD, 0:PAD],
                              in_=zsrc[0:64, 0:H * PAD]
                              .rearrange("p (a b) -> p a b", b=PAD))
            nc.sync.dma_start(out=xpr[0:64, PAD:HP - PAD, WP - PAD:WP],
                              in_=zsrc[0:64, 0:H * PAD]
                              .rearrange("p (a b) -> p a b", b=PAD))
            nc.sync.dma_start(out=xpr[64:128, PAD:HP - PAD, W:WP],
                              in_=zsrc[64:128, 0:H * 4]
                              .rearrange("p (a b) -> p a b", b=4))
            x3r = rr(xin[:, :]).rearrange("c (h w) -> c h w", h=H)
            RCHUNK = 32
            for k in range(H // RCHUNK):
                rs = slice(k * RCHUNK, (k + 1) * RCHUNK)
                nc.sync.dma_start(out=xpr[0:64, PAD + k * RCHUNK:PAD + (k + 1) * RCHUNK,
                                          PAD:PAD + W], in_=x3r[:, rs, :])
                nc.sync.dma_start(out=xpr[64:128, PAD + k * RCHUNK:PAD + (k + 1) * RCHUNK,
                                          0:W], in_=x3r[:, rs, :])

            ZC = 1024                  # z drain chunk: 2 tiles, 2 PSUM banks

            # ================= prologue (transient fp32 x + router) ==============
            with tc.tile_pool(name="stage", bufs=1) as stage, \
                 tc.tile_pool(name="ps_r", bufs=2, space="PSUM") as ps_r, \
                 tc.tile_pool(name="ps_mlp", bufs=1, space="PSUM") as ps_mlp:
                # exact fp32 x (router + GAP), partitions 0-63 only
                xps = stage.tile([C, HW], f32)
                for k in range(4):
                    nc.sync.dma_start(
                        out=xps[:, k * (HW // 4):(k + 1) * (HW // 4)],
                        in_=xin[:, k * (HW // 4):(k + 1) * (HW // 4)])

                # GAP: ACT copy pass with per-instruction accum_out, then reduce
                for k in range(8):
                    nc.scalar.activation(
                        out=gscr, in_=xps[:, k * 2048:(k + 1) * 2048],
                        func=AF.Copy, accum_out=gparts[:, k:k + 1])
                nc.vector.tensor_reduce(out=xg, in_=gparts, axis=AX.X, op=ALU.add)
                mlp1 = ps_mlp.tile([HID, 1], f32, tag="mlp")
                nc.tensor.matmul(mlp1, wG1, xg, start=True, stop=True)
                nc.scalar.activation(out=hrelu, in_=mlp1, func=AF.Relu)
                mlp2 = ps_mlp.tile([N_OPS, 1], f32, tag="mlp")
                nc.tensor.matmul(mlp2, wG2, hrelu, start=True, stop=True)
                nc.vector.tensor_add(biasv, mlp2, hbv)

                # spatial router stream (exact fp32); drain with bias; bounce to
                # DRAM for the pixel-major relayout
                for ch in range(HW // ZC):
                    zps = ps_r.tile([N_OPS, ZC], f32)
                    for j in range(ZC // TPX):
                        t = ch * (ZC // TPX) + j
                        nc.tensor.matmul(zps[:, j * TPX:(j + 1) * TPX], wR,
                                         xps[:, t * TPX:(t + 1) * TPX],
                                         start=True, stop=True)
                    zfl = zchunk.tile([N_OPS, ZC], f32)
                    nc.scalar.activation(out=zfl, in_=zps, func=AF.Identity,
                                         bias=biasv, scale=1.0)
                    nc.sync.dma_start(out=zscr[:, ch * ZC:(ch + 1) * ZC], in_=zfl)
                # load back pixel-major: zpm[p, n, b] = z[n, 128p + b]
                nc.sync.dma_start(
                    out=zpm,
                    in_=bass.AP(tensor=zscr.tensor, offset=zscr.offset,
                                ap=[[128, 128], [HW, N_OPS], [1, 128]]))

            # ---- softmax + top-2 (pixel-major, whole image) ---------------------
            nc.vector.tensor_reduce(out=m1, in_=overn(zpm), axis=AX.X, op=ALU.max)
            nc.vector.tensor_tensor(out=eqx, in0=zpm, in1=b5(m1), op=ALU.is_equal)
            nc.vector.scalar_tensor_tensor(out=eqx, in0=eqx, scalar=NEG_BIG,
                                           in1=zpm, op0=ALU.mult, op1=ALU.add)
            nc.vector.tensor_reduce(out=m2, in_=overn(eqx), axis=AX.X, op=ALU.max)
            nc.vector.scalar_tensor_tensor(out=em, in0=b5(m1), scalar=-1.0,
                                           in1=zpm, op0=ALU.mult, op1=ALU.add)
            nc.scalar.activation(out=em, in_=em, func=AF.Exp)
            nc.vector.tensor_reduce(out=zsum, in_=overn(em), axis=AX.X, op=ALU.add)
            nc.vector.tensor_tensor(out=eqx, in0=zpm, in1=b5(m2), op=ALU.is_ge)
            nc.vector.tensor_tensor(out=em, in0=em, in1=eqx, op=ALU.mult)
            nc.vector.tensor_reduce(out=e2, in_=overn(em), axis=AX.X, op=ALU.add)
            nc.vector.scalar_tensor_tensor(out=e2, in0=zsum, scalar=EPS,
                                           in1=e2, op0=ALU.mult, op1=ALU.add)
            nc.vector.reciprocal(out=e2, in_=e2)
            nc.vector.tensor_tensor(out=wpm, in0=em, in1=b5(e2), op=ALU.mult)
            # bounce w to DRAM flat layout
            nc.sync.dma_start(
                out=bass.AP(tensor=wscr.tensor, offset=wscr.offset,
                            ap=[[128, 128], [HW, N_OPS], [1, 128]]),
                in_=wpm)

            # ---- main loop ------------------------------------------------------
            with tc.tile_pool(name="wrep", bufs=2) as wrep, \
                 tc.tile_pool(name="gbuf", bufs=2) as gbuf, \
                 tc.tile_pool(name="outst", bufs=2) as outst, \
                 tc.tile_pool(name="ps_a", bufs=2, space="PSUM") as ps_a, \
                 tc.tile_pool(name="ps_bx", bufs=2, space="PSUM") as ps_bx, \
                 tc.tile_pool(name="ps_c", bufs=1, space="PSUM") as ps_c, \
                 tc.tile_pool(name="ps_w", bufs=1, space="PSUM") as ps_w, \
                 tc.tile_pool(name="ps_o", bufs=1, space="PSUM") as ps_o:
                WCH = 2048
                for ch in range(HW // WCH):
                    wfl = wchunk.tile([N_OPS, WCH], f32r)
                    nc.sync.dma_start(out=wfl,
                                      in_=rr(wscr[:, ch * WCH:(ch + 1) * WCH]))
                    ost = outst.tile([C, WCH], f32)
                    for j in range(WCH // TPX):
                        t = ch * (WCH // TPX) + j
                        h0 = t * ROWS_PER_TILE

                        def rhsAB(i):
                            kh = i if i < 3 else i - 3
                            p1 = 128 if i < 3 else 64
                            co = 1 if i < 3 else 2
                            return xpr[0:p1, 1 + kh + h0:1 + kh + h0 + ROWS_PER_TILE,
                                       co:co + W]

                        def rhsC(i):
                            kh = i if i < 3 else i - 3
                            p1 = 128 if i < 3 else 64
                            co = 0 if i < 3 else 4
                            return xpr[0:p1, 2 * kh + h0:2 * kh + h0 + ROWS_PER_TILE,
                                       co:co + W]

                        bankA = ps_a.tile([128, TPX], f32)
                        bankBX = ps_bx.tile([128, TPX], f32)
                        bankC = ps_c.tile([C, TPX], f32)
                        for i in range(6):
                            kk = 128 if i < 3 else 64
                            nc.tensor.matmul(bankA, wA[0:kk, i, :], rhsAB(i),
                                             start=(i == 0), stop=(i == 5))
                        for i in range(6):
                            kk = 128 if i < 3 else 64
                            nc.tensor.matmul(bankBX, wB[0:kk, i, :], rhsAB(i),
                                             start=(i == 0), stop=(i == 5))
                        for i in range(6):
                            kk = 128 if i < 3 else 64
                            nc.tensor.matmul(bankC, wC[0:kk, i, :], rhsC(i),
                                             start=(i == 0), stop=(i == 5))
                        # broadcast w maps across channel partitions (K=5 matmuls)
                        wsl = wfl[:, j * TPX:(j + 1) * TPX]
                        pw13 = ps_w.tile([128, TPX], f32, tag="pw13")
                        pw40 = ps_w.tile([128, TPX], f32, tag="pw40")
                        pw2 = ps_o.tile([64, TPX], f32, tag="pw2o")
                        nc.tensor.matmul(pw13, bc13, wsl, start=True, stop=True)
                        nc.tensor.matmul(pw40, bc40, wsl, start=True, stop=True)
                        nc.tensor.matmul(pw2, bc2, wsl, start=True, stop=True)
                        w13 = wrep.tile([128, TPX], f32, tag="w13")
                        w40 = wrep.tile([128, TPX], f32, tag="w40")
                        w2 = wrep.tile([64, TPX], f32, tag="w2")
                        nc.scalar.activation(out=w13, in_=pw13, func=AF.Copy)
                        nc.scalar.activation(out=w40, in_=pw40, func=AF.Copy)
                        nc.scalar.activation(out=w2, in_=pw2, func=AF.Copy)
                        # mix: g = (f + bias) * w    (outputs rounded to fp32r)
                        gA = gbuf.tile([128, TPX], f32r, tag="gA")
                        gBX = gbuf.tile([128, TPX], f32r, tag="gBX")
                        gC = gbuf.tile([C, TPX], f32r, tag="gC")
                        nc.vector.scalar_tensor_tensor(out=gA, in0=bankA, scalar=bias13,
                                                       in1=w13, op0=ALU.add,
                                                       op1=ALU.mult)
                        nc.vector.scalar_tensor_tensor(out=gBX, in0=bankBX,
                                                       scalar=bias40, in1=w40,
                                                       op0=ALU.add, op1=ALU.mult)
                        nc.vector.scalar_tensor_tensor(out=gC, in0=bankC,
                                                       scalar=bias2, in1=w2,
                                                       op0=ALU.add, op1=ALU.mult)
                        # sum the 5 terms in PSUM via identity matmuls
                        po = ps_o.tile([64, TPX], f32, tag="pw2o")
                        nc.tensor.matmul(po, s128, gA, start=True, stop=False)
                        nc.tensor.matmul(po, s128, gBX, start=False, stop=False)
                        nc.tensor.matmul(po, s64, gC, start=False, stop=True)
                        nc.vector.tensor_copy(ost[:, j * TPX:(j + 1) * TPX], po)
                    nc.sync.dma_start(out=yout[:, ch * WCH:(ch + 1) * WCH], in_=ost)


    nc.compile()
    return nc


def _get_program():
    if "nc" not in _CACHE:
        _CACHE["nc"] = _build_program()
    return _CACHE["nc"]


def _run(inputs, **spmd_kwargs):
    x = np.ascontiguousarray(np.asarray(inputs["x"], np.float32))
    consts = _host_consts(**{k: inputs[k] for k in
                             ["w3", "b3", "wd", "bd", "wdw", "bdw",
                              "gr_w1", "gr_w2", "gr_b2", "sr_w", "sr_b"]})
    nc = _get_program()

    from concourse.bass_utils import run_bass_kernel_spmd
    in_maps = []
    for i in range(B):
        m = dict(consts)
        m["xin"] = np.ascontiguousarray(x[i].reshape(C, HW))
        in_maps.append(m)
    res = run_bass_kernel_spmd(nc, in_maps, core_ids=list(range(B)), **spmd_kwargs)
    out = np.stack([res.results[i]["yout"].reshape(C, H, W) for i in range(B)])
    return out.astype(np.float32), res


def kernel(**inputs):
    out, _ = _run(inputs)
    return out



# revision 6
# speedup vs baseline: 1.1278x; 1.1278x over previous
# Trainium2 Bass kernel for DMOR (dynamic mixture-of-operators routing).
#
# Reference computation (per image):
#   op_feats = [x, conv3x3(x), conv3x3_dilated2(x), avgpool3x3(x), dwconv3x3(x)]
#   z = spatial_router(x) + global_router(GAP(x))          # [5, H, W]
#   w = softmax(z, axis=0); top-2 mask + renormalize (eps=1e-6)
#   out = sum_n w_n * op_feats_n
#
# Sharding: data-parallel over batch B=8 across 8 NeuronCores (1 image/core),
# weights replicated. One SPMD program; per-core in_maps differ only in x.
#
# Device structure (per core, x:[64,128,128] -> out:[64,128,128]):
#   - All stencil ops become 9-shift accumulating PSUM matmuls in float32r
#     (~1.7e-4 rel err, ~1 col/cycle). fp32r tolerates only plain full-array
#     matmuls (no tile_position), so shift-pairs are packed into K=128 by
#     storing x twice: partitions 0-63 hold padded x, partitions 64-127 hold
#     the same image shifted left by 2 columns. 3 PSUM groups:
#       bankA[128,512] = [conv3x3 | avgpool]   (avg = (1/9)*I diagonal)
#       bankBX[128,512] = [dwconv | identity]  (identity = I at center shift)
#       bankC[64,512]  = dilated conv
#   - Router z: exact fp32 matmuls (top-2 selection is discontinuous; z gaps
#     go down to ~1e-7). GAP via an ACT copy pass with accum_out; tiny MLP on
#     the PE; the data-dependent global bias is fused into the z drain.
#   - z is moved to a pixel-major layout [128,5,128] via a DRAM bounce
#     (partition-crossing SBUF->SBUF DMAs are miscompiled, measured), then
#     softmax/top-2 runs as ~13 whole-image DVE/ACT ops.
#   - w returns flat via DRAM; K=5 ones-matmuls broadcast the three needed
#     (map-pair) layouts across channel partitions; the mix is 3 fused
#     (f + bias) * w ops per tile; the 5-term sum accumulates in PSUM via
#     identity matmuls.
import numpy as np

B, C, H, W = 8, 64, 128, 128
HW = H * W
N_OPS = 5
HID = 16
PAD = 2
HP, WP = H + 2 * PAD, W + 2 * PAD  # 132, 132
TPX = 512                          # pixels per tile (4 image rows)
NT = HW // TPX                     # 32 tiles
ROWS_PER_TILE = TPX // W           # 4
EPS = 1e-6
NEG_BIG = -1.0e30

_CACHE = {}


def _host_consts(w3, b3, wd, bd, wdw, bdw, gr_w1, gr_w2, gr_b2, sr_w, sr_b):
    """Precompute all stationary matmul operands / bias vectors in numpy.

    Matmul i in 0..5 per op group: i<3 are K=128 shift-pairs (kh=i, partition
    half 0 = col-shift kw0, half 1 = kw1 via the +2-shifted x copy); i>=3 are
    K=64 unpaired (kh=i-3, remaining kw)."""
    f32 = np.float32
    w3 = np.asarray(w3, f32); wd = np.asarray(wd, f32); wdw = np.asarray(wdw, f32)
    eye = np.eye(C, dtype=f32)
    avg = eye * f32(1.0 / 9.0)

    A = np.zeros((128, 6, 128), f32)
    Bd = np.zeros((128, 6, 128), f32)
    Cd = np.zeros((128, 6, 64), f32)
    for kh in range(3):
        # paired: conv3/dw cols (kw=0 | kw=2); dilated cols (kw=0 | kw=1)
        A[0:64, kh, 0:64] = w3[:, :, kh, 0].T
        A[64:128, kh, 0:64] = w3[:, :, kh, 2].T
        A[0:64, kh, 64:128] = avg
        A[64:128, kh, 64:128] = avg
        Bd[0:64, kh, 0:64] = eye * wdw[:, 0, kh, 0][None, :]
        Bd[64:128, kh, 0:64] = eye * wdw[:, 0, kh, 2][None, :]
        Cd[0:64, kh, :] = wd[:, :, kh, 0].T
        Cd[64:128, kh, :] = wd[:, :, kh, 1].T
        # unpaired: conv3/dw kw=1; dilated kw=2
        A[0:64, 3 + kh, 0:64] = w3[:, :, kh, 1].T
        A[0:64, 3 + kh, 64:128] = avg
        Bd[0:64, 3 + kh, 0:64] = eye * wdw[:, 0, kh, 1][None, :]
        Cd[0:64, 3 + kh, :] = wd[:, :, kh, 2].T
    Bd[0:64, 4, 64:128] = eye  # identity op = center shift (kh=1, kw=1)

    R = np.ascontiguousarray(np.asarray(sr_w, f32)[:, :, 0, 0].T)          # [C, 5]
    G1 = np.ascontiguousarray((np.asarray(gr_w1, f32)[:, :, 0, 0] / HW).T)  # [C, HID]
    G2 = np.ascontiguousarray(np.asarray(gr_w2, f32)[:, :, 0, 0].T)        # [HID, 5]
    HBIAS = (np.asarray(sr_b, f32) + np.asarray(gr_b2, f32)).reshape(N_OPS, 1)

    SUM128 = np.ascontiguousarray(np.concatenate([eye, eye], axis=0))  # [128, 64]

    def bc(n_lo, n_hi=None):
        m = 128 if n_hi is not None else 64
        out = np.zeros((N_OPS, m), f32)
        out[n_lo, 0:64] = 1.0
        if n_hi is not None:
            out[n_hi, 64:128] = 1.0
        return out

    BIAS13 = np.concatenate([np.asarray(b3, f32), np.zeros(C, f32)]).reshape(128, 1)
    BIAS40 = np.concatenate([np.asarray(bdw, f32), np.zeros(C, f32)]).reshape(128, 1)
    BIAS2 = np.asarray(bd, f32).reshape(C, 1)

    return {
        "cA": np.ascontiguousarray(A.reshape(128, 6 * 128)),
        "cB": np.ascontiguousarray(Bd.reshape(128, 6 * 128)),
        "cC": np.ascontiguousarray(Cd.reshape(128, 6 * 64)),
        "cR": R, "cG1": G1, "cG2": G2, "cHB": HBIAS,
        "cSUM128": SUM128, "cSUM64": eye,
        "cBC13": bc(1, 3), "cBC40": bc(4, 0), "cBC2": bc(2),
        "cBIAS13": BIAS13, "cBIAS40": BIAS40, "cBIAS2": BIAS2,
    }


def _host_pad(x_img):
    """Build the dual-shift padded image [128, HP, WP] on the host:
    partitions 0-63 = zero-padded x; 64-127 = same, shifted left 2 cols."""
    xp = np.zeros((128, HP, WP), np.float32)
    xp[0:C, PAD:PAD + H, PAD:PAD + W] = x_img
    xp[C:128, PAD:PAD + H, 0:W] = x_img
    return np.ascontiguousarray(xp.reshape(128, HP * WP))


def _build_program(nrep=1):
    import concourse.bass as bass
    import concourse.bacc as bacc
    import concourse.tile as tile
    import concourse.mybir as mybir
    from contextlib import ExitStack

    dt = mybir.dt
    f32 = dt.float32
    f32r = dt.float32r
    AF = mybir.ActivationFunctionType
    ALU = mybir.AluOpType
    AX = mybir.AxisListType

    nc = bacc.Bacc("TRN2", target_bir_lowering=False, debug=False)

    xin = nc.dram_tensor("xin", [C, HW], f32, kind="ExternalInput")
    xpad = nc.dram_tensor("xpad", [128, HP * WP], f32, kind="ExternalInput")
    dr = {}
    for name, shape in [
        ("cA", [128, 6 * 128]), ("cB", [128, 6 * 128]), ("cC", [128, 6 * 64]),
        ("cR", [C, N_OPS]), ("cG1", [C, HID]), ("cG2", [HID, N_OPS]),
        ("cHB", [N_OPS, 1]), ("cSUM128", [128, 64]), ("cSUM64", [64, 64]),
        ("cBC13", [N_OPS, 128]), ("cBC40", [N_OPS, 128]), ("cBC2", [N_OPS, 64]),
        ("cBIAS13", [128, 1]), ("cBIAS40", [128, 1]), ("cBIAS2", [64, 1]),
    ]:
        dr[name] = nc.dram_tensor(name, shape, f32, kind="ExternalInput")
    yout = nc.dram_tensor("yout", [C, HW], f32, kind="ExternalOutput")

    def b5(t):
        # broadcast a [128,128] tile across the 5-map free dim -> [128,5,128]
        return bass.AP(tensor=t.tensor, offset=t.offset,
                       ap=[list(t.ap[0]), [0, N_OPS], list(t.ap[1])])

    def rr(ap):
        return ap.bitcast(f32r)

    with tile.TileContext(nc) as tc, ExitStack() as ctx:
        consts = ctx.enter_context(tc.tile_pool(name="consts", bufs=1))
        xpool = ctx.enter_context(tc.tile_pool(name="xp", bufs=1))
        zpool = ctx.enter_context(tc.tile_pool(name="z", bufs=1))
        gaps = ctx.enter_context(tc.tile_pool(name="gaps", bufs=1))
        zchunk = ctx.enter_context(tc.tile_pool(name="zchunk", bufs=2))
        wchunk = ctx.enter_context(tc.tile_pool(name="wchunk", bufs=3))
        drpool = ctx.enter_context(tc.tile_pool(name="drbounce", bufs=1,
                                                space="DRAM"))

        # ---- constant tiles -------------------------------------------------
        wA = consts.tile([128, 6, 128], f32r)
        wB = consts.tile([128, 6, 128], f32r)
        wC = consts.tile([128, 6, 64], f32r)
        s128 = consts.tile([128, 64], f32r)
        s64 = consts.tile([C, 64], f32r)
        bc13 = consts.tile([N_OPS, 128], f32r)
        bc40 = consts.tile([N_OPS, 128], f32r)
        bc2 = consts.tile([N_OPS, 64], f32r)
        wR = consts.tile([C, N_OPS], f32)
        wG1 = consts.tile([C, HID], f32)
        wG2 = consts.tile([HID, N_OPS], f32)
        hbv = consts.tile([N_OPS, 1], f32)
        bias13 = consts.tile([128, 1], f32)
        bias40 = consts.tile([128, 1], f32)
        bias2 = consts.tile([C, 1], f32)

        nc.sync.dma_start(out=wA,
                          in_=rr(dr["cA"][:, :]).rearrange("c (s m) -> c s m", s=6))
        nc.sync.dma_start(out=wB,
                          in_=rr(dr["cB"][:, :]).rearrange("c (s m) -> c s m", s=6))
        nc.sync.dma_start(out=wC,
                          in_=rr(dr["cC"][:, :]).rearrange("c (s m) -> c s m", s=6))
        nc.sync.dma_start(out=s128, in_=rr(dr["cSUM128"][:, :]))
        nc.sync.dma_start(out=s64, in_=rr(dr["cSUM64"][:, :]))
        nc.sync.dma_start(out=bc13, in_=rr(dr["cBC13"][:, :]))
        nc.sync.dma_start(out=bc40, in_=rr(dr["cBC40"][:, :]))
        nc.sync.dma_start(out=bc2, in_=rr(dr["cBC2"][:, :]))
        for t, name in [(wR, "cR"), (wG1, "cG1"), (wG2, "cG2"), (hbv, "cHB"),
                        (bias13, "cBIAS13"), (bias40, "cBIAS40"),
                        (bias2, "cBIAS2")]:
            nc.sync.dma_start(out=t, in_=dr[name][:, :])

        zpm = zpool.tile([128, N_OPS, 128], f32)
        rep_pools = (consts, xpool, zpool, gaps, zchunk, wchunk, drpool)
        gscr = gaps.tile([C, 2048], f32)
        gparts = gaps.tile([C, 8], f32)
        xg = gaps.tile([C, 1], f32)
        hrelu = gaps.tile([HID, 1], f32)
        biasv = gaps.tile([N_OPS, 1], f32)
        zscr = drpool.tile([N_OPS, HW], f32)
        wscr = drpool.tile([N_OPS, HW], f32)


        m1 = zpool.tile([128, 128], f32)
        m2 = zpool.tile([128, 128], f32)
        zsum = zpool.tile([128, 128], f32)
        e2 = zpool.tile([128, 128], f32)
        eqx = zpool.tile([128, N_OPS, 128], f32)
        em = zpool.tile([128, N_OPS, 128], f32)
        wpm = zpool.tile([128, N_OPS, 128], f32)

        def overn(t):
            return t[:, :, :].rearrange("p n b -> p b n")

        for _rep in range(nrep):
            # -- fp32r x, two copies (host-prepadded): half0 = padded x,
            #    half1 = same shifted left 2 cols. Few large contiguous DMAs.
            xpr = xpool.tile([128, HP, WP], f32r)
            xsrc = rr(xpad[:, :]).rearrange("p (h w) -> p h w", h=HP)
            XCH = 33                  # rows per load chunk (132 = 4*33)
            for k in range(HP // XCH):
                nc.sync.dma_start(out=xpr[:, k * XCH:(k + 1) * XCH, :],
                                  in_=xsrc[:, k * XCH:(k + 1) * XCH, :])

            ZC = 1024                  # z drain chunk: 2 tiles, 2 PSUM banks
            XPS = HW // 4              # fp32 x chunk (one tile each)

            # ================= prologue (transient fp32 x + router) ==============
            with tc.tile_pool(name="stage", bufs=1) as stage, \
                 tc.tile_pool(name="ps_r", bufs=2, space="PSUM") as ps_r, \
                 tc.tile_pool(name="ps_mlp", bufs=1, space="PSUM") as ps_mlp:
                # exact fp32 x (router + GAP), partitions 0-63 only; 4
                # independent tiles so the router starts after the first lands
                xps = [stage.tile([C, XPS], f32, tag=f"xps{k}", name=f"xps{k}")
                       for k in range(4)]
                for k in range(4):
                    nc.sync.dma_start(out=xps[k],
                                      in_=xin[:, k * XPS:(k + 1) * XPS])

                # GAP: ACT copy pass with per-instruction accum_out, then reduce
                for k in range(8):
                    nc.scalar.activation(
                        out=gscr, in_=xps[k // 2][:, (k % 2) * 2048:(k % 2 + 1) * 2048],
                        func=AF.Copy, accum_out=gparts[:, k:k + 1])
                nc.vector.tensor_reduce(out=xg, in_=gparts, axis=AX.X, op=ALU.add)
                mlp1 = ps_mlp.tile([HID, 1], f32, tag="mlp")
                nc.tensor.matmul(mlp1, wG1, xg, start=True, stop=True)
                nc.scalar.activation(out=hrelu, in_=mlp1, func=AF.Relu)
                mlp2 = ps_mlp.tile([N_OPS, 1], f32, tag="mlp")
                nc.tensor.matmul(mlp2, wG2, hrelu, start=True, stop=True)
                nc.vector.tensor_add(biasv, mlp2, hbv)

                # spatial router stream (exact fp32); drain with bias; bounce to
                # DRAM for the pixel-major relayout
                for ch in range(HW // ZC):
                    zps = ps_r.tile([N_OPS, ZC], f32)
                    for j in range(ZC // TPX):
                        px0 = ch * ZC + j * TPX
                        nc.tensor.matmul(zps[:, j * TPX:(j + 1) * TPX], wR,
                                         xps[px0 // XPS][:, px0 % XPS:px0 % XPS + TPX],
                                         start=True, stop=True)
                    zfl = zchunk.tile([N_OPS, ZC], f32)
                    nc.scalar.activation(out=zfl, in_=zps, func=AF.Identity,
                                         bias=biasv, scale=1.0)
                    nc.sync.dma_start(out=zscr[:, ch * ZC:(ch + 1) * ZC], in_=zfl)
                # load back pixel-major: zpm[p, n, b] = z[n, 128p + b]
                nc.sync.dma_start(
                    out=zpm,
                    in_=bass.AP(tensor=zscr.tensor, offset=zscr.offset,
                                ap=[[128, 128], [HW, N_OPS], [1, 128]]))

            # ---- softmax + top-2 (pixel-major, whole image) ---------------------
            nc.vector.tensor_reduce(out=m1, in_=overn(zpm), axis=AX.X, op=ALU.max)
            nc.vector.tensor_tensor(out=eqx, in0=zpm, in1=b5(m1), op=ALU.is_equal)
            nc.vector.scalar_tensor_tensor(out=eqx, in0=eqx, scalar=NEG_BIG,
                                           in1=zpm, op0=ALU.mult, op1=ALU.add)
            nc.vector.tensor_reduce(out=m2, in_=overn(eqx), axis=AX.X, op=ALU.max)
            nc.vector.scalar_tensor_tensor(out=em, in0=b5(m1), scalar=-1.0,
                                           in1=zpm, op0=ALU.mult, op1=ALU.add)
            nc.scalar.activation(out=em, in_=em, func=AF.Exp)
            nc.vector.tensor_reduce(out=zsum, in_=overn(em), axis=AX.X, op=ALU.add)
            nc.vector.tensor_tensor(out=eqx, in0=zpm, in1=b5(m2), op=ALU.is_ge)
            nc.vector.tensor_tensor(out=em, in0=em, in1=eqx, op=ALU.mult)
            nc.vector.tensor_reduce(out=e2, in_=overn(em), axis=AX.X, op=ALU.add)
            nc.vector.scalar_tensor_tensor(out=e2, in0=zsum, scalar=EPS,
                                           in1=e2, op0=ALU.mult, op1=ALU.add)
            nc.vector.reciprocal(out=e2, in_=e2)
            nc.vector.tensor_tensor(out=wpm, in0=em, in1=b5(e2), op=ALU.mult)
            # bounce w to DRAM flat layout
            nc.sync.dma_start(
                out=bass.AP(tensor=wscr.tensor, offset=wscr.offset,
                            ap=[[128, 128], [HW, N_OPS], [1, 128]]),
                in_=wpm)

            # ---- main loop ------------------------------------------------------
            with tc.tile_pool(name="wrep", bufs=2) as wrep, \
                 tc.tile_pool(name="gbuf", bufs=2) as gbuf, \
                 tc.tile_pool(name="outst", bufs=2) as outst, \
                 tc.tile_pool(name="ps_a", bufs=2, space="PSUM") as ps_a, \
                 tc.tile_pool(name="ps_bx", bufs=2, space="PSUM") as ps_bx, \
                 tc.tile_pool(name="ps_c", bufs=1, space="PSUM") as ps_c, \
                 tc.tile_pool(name="ps_w", bufs=1, space="PSUM") as ps_w, \
                 tc.tile_pool(name="ps_o", bufs=1, space="PSUM") as ps_o:
                WCH = 2048
                for ch in range(HW // WCH):
                    wfl = wchunk.tile([N_OPS, WCH], f32r)
                    nc.sync.dma_start(out=wfl,
                                      in_=rr(wscr[:, ch * WCH:(ch + 1) * WCH]))
                    ost = outst.tile([C, WCH], f32)
                    for j in range(WCH // TPX):
                        t = ch * (WCH // TPX) + j
                        h0 = t * ROWS_PER_TILE

                        def rhsAB(i):
                            kh = i if i < 3 else i - 3
                            p1 = 128 if i < 3 else 64
                            co = 1 if i < 3 else 2
                            return xpr[0:p1, 1 + kh + h0:1 + kh + h0 + ROWS_PER_TILE,
                                       co:co + W]

                        def rhsC(i):
                            kh = i if i < 3 else i - 3
                            p1 = 128 if i < 3 else 64
                            co = 0 if i < 3 else 4
                            return xpr[0:p1, 2 * kh + h0:2 * kh + h0 + ROWS_PER_TILE,
                                       co:co + W]

                        bankA = ps_a.tile([128, TPX], f32)
                        bankBX = ps_bx.tile([128, TPX], f32)
                        bankC = ps_c.tile([C, TPX], f32)
                        for i in range(6):
                            kk = 128 if i < 3 else 64
                            nc.tensor.matmul(bankA, wA[0:kk, i, :], rhsAB(i),
                                             start=(i == 0), stop=(i == 5))
                        for i in range(6):
                            kk = 128 if i < 3 else 64
                            nc.tensor.matmul(bankBX, wB[0:kk, i, :], rhsAB(i),
                                             start=(i == 0), stop=(i == 5))
                        for i in range(6):
                            kk = 128 if i < 3 else 64
                            nc.tensor.matmul(bankC, wC[0:kk, i, :], rhsC(i),
                                             start=(i == 0), stop=(i == 5))
                        # broadcast w maps across channel partitions (K=5 matmuls)
                        wsl = wfl[:, j * TPX:(j + 1) * TPX]
                        pw13 = ps_w.tile([128, TPX], f32, tag="pw13")
                        pw40 = ps_w.tile([128, TPX], f32, tag="pw40")
                        pw2 = ps_o.tile([64, TPX], f32, tag="pw2o")
                        nc.tensor.matmul(pw13, bc13, wsl, start=True, stop=True)
                        nc.tensor.matmul(pw40, bc40, wsl, start=True, stop=True)
                        nc.tensor.matmul(pw2, bc2, wsl, start=True, stop=True)
                        w13 = wrep.tile([128, TPX], f32, tag="w13")
                        w40 = wrep.tile([128, TPX], f32, tag="w40")
                        w2 = wrep.tile([64, TPX], f32, tag="w2")
                        nc.scalar.activation(out=w13, in_=pw13, func=AF.Copy)
                        nc.scalar.activation(out=w40, in_=pw40, func=AF.Copy)
                        nc.scalar.activation(out=w2, in_=pw2, func=AF.Copy)
                        # mix: g = (f + bias) * w    (outputs rounded to fp32r)
                        gA = gbuf.tile([128, TPX], f32r, tag="gA")
                        gBX = gbuf.tile([128, TPX], f32r, tag="gBX")
                        gC = gbuf.tile([C, TPX], f32r, tag="gC")
                        nc.vector.scalar_tensor_tensor(out=gA, in0=bankA, scalar=bias13,
                                                       in1=w13, op0=ALU.add,
                                                       op1=ALU.mult)
                        nc.vector.scalar_tensor_tensor(out=gBX, in0=bankBX,
                                                       scalar=bias40, in1=w40,
                                                       op0=ALU.add, op1=ALU.mult)
                        nc.vector.scalar_tensor_tensor(out=gC, in0=bankC,
                                                       scalar=bias2, in1=w2,
                                                       op0=ALU.add, op1=ALU.mult)
                        # sum the 5 terms in PSUM via identity matmuls
                        po = ps_o.tile([64, TPX], f32, tag="pw2o")
                        nc.tensor.matmul(po, s128, gA, start=True, stop=False)
                        nc.tensor.matmul(po, s128, gBX, start=False, stop=False)
                        nc.tensor.matmul(po, s64, gC, start=False, stop=True)
                        nc.vector.tensor_copy(ost[:, j * TPX:(j + 1) * TPX], po)
                    nc.sync.dma_start(out=yout[:, ch * WCH:(ch + 1) * WCH], in_=ost)


    nc.compile()
    return nc


def _get_program():
    if "nc" not in _CACHE:
        _CACHE["nc"] = _build_program()
    return _CACHE["nc"]


def _run(inputs, **spmd_kwargs):
    x = np.ascontiguousarray(np.asarray(inputs["x"], np.float32))
    consts = _host_consts(**{k: inputs[k] for k in
                             ["w3", "b3", "wd", "bd", "wdw", "bdw",
                              "gr_w1", "gr_w2", "gr_b2", "sr_w", "sr_b"]})
    nc = _get_program()

    from concourse.bass_utils import run_bass_kernel_spmd
    in_maps = []
    for i in range(B):
        m = dict(consts)
        m["xin"] = np.ascontiguousarray(x[i].reshape(C, HW))
        m["xpad"] = _host_pad(x[i])
        in_maps.append(m)
    res = run_bass_kernel_spmd(nc, in_maps, core_ids=list(range(B)), **spmd_kwargs)
    out = np.stack([res.results[i]["yout"].reshape(C, H, W) for i in range(B)])
    return out.astype(np.float32), res


def kernel(**inputs):
    out, _ = _run(inputs)
    return out

